# revision 1
# baseline (speedup 1.0000x reference)
"""GaussMemoryStep Trainium2 kernel.

Math (reference):
  X_ri = rfft(x)[1:257] as [real, imag]            # [B,T,512]
  q,k,v = X_ri @ {wq,wk,wv}.T                      # [B,T,512]
  scores = q @ k^T                                 # [B,T,T]
  weights[i,j] = decay^(j-i-1) for j>i else 0      # future-looking decay band
  retrieved = (scores*weights) @ v
  out = irfft(embed(retrieved @ wo)) * out_scale   # [B,T,8192]

Kernel strategy (8 cores, core c handles sample b=c//2, half h=c%2):
  - rfft folded into one [8192,512] DFT matmul against resident F; X_riT kept
    on-chip; small 512x512 projections; banded attention (decay^o < 1e-11
    beyond W=256 offsets, so only the next 768 keys matter per 512-query
    block); synthesis via Wo_eff = wo @ G (irfft matrix).
  - decay handled exactly via: v rows pre-scaled by d^j (per-partition scale),
    output rows post-scaled by d^-(i+1) * out_scale; the scores/PV matmuls
    then need only a causal-band mask.
  - all matmuls in float32r (tf32-like, full PE rate at N>=256).
  - x transposed on-chip via PE transpose (exact for fp32).
"""
import numpy as np
import concourse.bacc as bacc
import concourse.mybir as mybir
import concourse.tile as tile
from concourse import bass_utils

B, T, V, C, NF = 4, 2048, 8192, 512, 256
M2 = 2 * NF            # 512 = ri-concat width (== C)
QR = 1024              # query rows per core
KVR = 1280             # kv rows per core (query rows + band W; h=1 zero-padded)
KVP = 1536             # padded X_riT/kT free size (3*512)
W = 256                # decay band width
NCORES = 8

F32 = mybir.dt.float32
F32R = mybir.dt.float32r

_CACHE = {}


def _build():
    nc = bacc.Bacc("TRN2", target_bir_lowering=False, debug=False)

    xs = nc.dram_tensor("xs", [KVR, V], F32, kind="ExternalInput").ap()
    f = nc.dram_tensor("f", [V // 2, M2], F32R, kind="ExternalInput").ap()
    wqt = nc.dram_tensor("wqt", [M2, C], F32R, kind="ExternalInput").ap()
    wkt = nc.dram_tensor("wkt", [M2, C], F32R, kind="ExternalInput").ap()
    wvt = nc.dram_tensor("wvt", [M2, C], F32R, kind="ExternalInput").ap()
    woe = nc.dram_tensor("woe", [C, V], F32R, kind="ExternalInput").ap()
    idn = nc.dram_tensor("idn", [128, 128], F32R, kind="ExternalInput").ap()
    masks = nc.dram_tensor("masks", [128, 4 * 512], F32, kind="ExternalInput").ap()
    dvecs = nc.dram_tensor("dvecs", [128, 10], F32, kind="ExternalInput").ap()
    ovecs = nc.dram_tensor("ovecs", [128, 8], F32, kind="ExternalInput").ap()
    out = nc.dram_tensor("out", [QR, V], F32, kind="ExternalOutput").ap()

    RGS = [(0, 512), (512, 512), (1024, 256)]

    with tile.TileContext(nc) as tc:
      with tc.tile_pool(name="consts", bufs=1) as cp, \
           tc.tile_pool(name="xri", bufs=1) as xrip:
        ident = cp.tile([128, 128], F32R, tag="idn")
        nc.sync.dma_start(ident[:], idn[:])
        dvec_t = cp.tile([128, 10], F32, tag="dv")
        nc.sync.dma_start(dvec_t[:], dvecs[:])
        ovec_t = cp.tile([128, 8], F32, tag="ov")
        nc.sync.dma_start(ovec_t[:], ovecs[:])
        zt = cp.tile([128, 256], F32, tag="zt")
        nc.vector.memset(zt[:], 0.0)

        xri = [xrip.tile([128, KVP], F32R, tag=f"xri{mc}", name=f"xri{mc}")
               for mc in range(4)]
        for mc in range(4):
            nc.vector.tensor_copy(xri[mc][:, KVR:KVP], zt[:])

        # ---- Phase 1: rfft-fold + X_riT ----
        # xp/xm = x[:, :4096] +/- x[:, 4096:]; even-k spectrum needs only xp,
        # odd-k only xm, halving the DFT contraction. F2 = F[:4096, perm] with
        # channels reordered [even-re|even-im|odd-re|odd-im] (weights permuted
        # to match on host).
        with tc.tile_pool(name="fp", bufs=1) as fp, \
             tc.tile_pool(name="xin", bufs=3) as xin, \
             tc.tile_pool(name="xpm", bufs=3) as xpm, \
             tc.tile_pool(name="xtp", bufs=4) as xtp, \
             tc.tile_pool(name="ps1", bufs=1, space="PSUM") as ps1, \
             tc.tile_pool(name="pst", bufs=3, space="PSUM") as pst:
            f_t = []
            for vc in range(32):
                ft = fp.tile([128, M2], F32R, tag=f"f{vc}", name=f"f{vc}")
                nc.sync.dma_start(ft[:], f[128 * vc:128 * (vc + 1), :])
                f_t.append(ft)

            for (r0, rlen) in RGS:
                nrs = rlen // 128
                psx = [ps1.tile([128, rlen], F32, tag=f"px{mc}", name=f"px{mc}")
                       for mc in range(4)]
                for vg in range(8):  # v'-groups of 512 within [0, 4096)
                    xp_t, xm_t = [], []
                    for rs in range(nrs):
                        xlo = xin.tile([128, 512], F32, tag=f"xl{rs}", name=f"xl{rs}")
                        nc.sync.dma_start(
                            xlo[:], xs[r0 + 128 * rs:r0 + 128 * (rs + 1),
                                       512 * vg:512 * (vg + 1)])
                        xhi = xin.tile([128, 512], F32, tag=f"xh{rs}", name=f"xh{rs}")
                        nc.sync.dma_start(
                            xhi[:], xs[r0 + 128 * rs:r0 + 128 * (rs + 1),
                                       4096 + 512 * vg:4096 + 512 * (vg + 1)])
                        xp = xpm.tile([128, 512], F32R, tag=f"xp{rs}", name=f"xp{rs}")
                        nc.vector.tensor_add(xp[:], xlo[:], xhi[:])
                        xm = xpm.tile([128, 512], F32R, tag=f"xm{rs}", name=f"xm{rs}")
                        nc.vector.tensor_sub(xm[:], xlo[:], xhi[:])
                        xp_t.append(xp)
                        xm_t.append(xm)
                    for v4 in range(4):
                        vc = 4 * vg + v4
                        for half, xsrc in ((0, xp_t), (1, xm_t)):
                            ptr = pst.tile([128, rlen], F32R, tag="ptr")
                            for rs in range(nrs):
                                nc.tensor.transpose(
                                    ptr[:, 128 * rs:128 * (rs + 1)],
                                    xsrc[rs][:, 128 * v4:128 * (v4 + 1)],
                                    ident[:])
                            xt = xtp.tile([128, rlen], F32R, tag="xt")
                            nc.vector.tensor_copy(xt[:], ptr[:])
                            for mh in range(2):
                                mc = 2 * half + mh
                                nc.tensor.matmul(
                                    psx[mc][:],
                                    f_t[vc][:, 128 * mc:128 * (mc + 1)],
                                    xt[:], start=(vc == 0), stop=(vc == 31),
                                    skip_group_check=True)
                for mc in range(4):
                    nc.vector.tensor_copy(xri[mc][:, r0:r0 + rlen], psx[mc][:])

        # ---- Phase 2: qT, kT, vsc ----
        with tc.tile_pool(name="prj", bufs=1) as prj, \
             tc.tile_pool(name="wp", bufs=1) as wp:
            qt = [prj.tile([128, QR], F32R, tag=f"qt{cc}", name=f"qt{cc}")
                  for cc in range(4)]
            kt = [prj.tile([128, KVP], F32R, tag=f"kt{cc}", name=f"kt{cc}")
                  for cc in range(4)]
            vsc = [prj.tile([128, C], F32R, tag=f"vs{rc}", name=f"vs{rc}")
                   for rc in range(10)]
            rt = [prj.tile([128, QR], F32R, tag=f"rt{cc}", name=f"rt{cc}")
                  for cc in range(4)]

            with tc.tile_pool(name="ps2", bufs=2, space="PSUM") as ps2:
                wq_t, wk_t, wv_t = [], [], []
                for wname, dram, lst in (("wq", wqt, wq_t), ("wk", wkt, wk_t),
                                         ("wv", wvt, wv_t)):
                    for mc in range(4):
                        wt = wp.tile([128, C], F32R, tag=f"{wname}{mc}",
                                     name=f"{wname}{mc}")
                        nc.sync.dma_start(wt[:], dram[128 * mc:128 * (mc + 1), :])
                        lst.append(wt)

                for cc in range(4):
                    for rtile in range(2):
                        ps = ps2.tile([128, 512], F32, tag="pp")
                        for mc in range(4):
                            nc.tensor.matmul(
                                ps[:], wq_t[mc][:, 128 * cc:128 * (cc + 1)],
                                xri[mc][:, 512 * rtile:512 * (rtile + 1)],
                                start=(mc == 0), stop=(mc == 3),
                                skip_group_check=True)
                        nc.vector.tensor_copy(
                            qt[cc][:, 512 * rtile:512 * (rtile + 1)], ps[:])
                    for rtile in range(3):
                        ps = ps2.tile([128, 512], F32, tag="pp")
                        for mc in range(4):
                            nc.tensor.matmul(
                                ps[:], wk_t[mc][:, 128 * cc:128 * (cc + 1)],
                                xri[mc][:, 512 * rtile:512 * (rtile + 1)],
                                start=(mc == 0), stop=(mc == 3),
                                skip_group_check=True)
                        nc.vector.tensor_copy(
                            kt[cc][:, 512 * rtile:512 * (rtile + 1)], ps[:])
                for rc in range(10):
                    ps = ps2.tile([128, 512], F32, tag="pp")
                    for mc in range(4):
                        nc.tensor.matmul(
                            ps[:], xri[mc][:, 1 + 128 * rc:129 + 128 * rc],
                            wv_t[mc][:], start=(mc == 0), stop=(mc == 3),
                            skip_group_check=True)
                    nc.vector.tensor_scalar_mul(vsc[rc][:], ps[:],
                                                dvec_t[:, rc:rc + 1])

            # ---- Phase 3: banded decay attention ----
            with tc.tile_pool(name="mk", bufs=1) as mk, \
                 tc.tile_pool(name="ssb", bufs=8) as ssbp, \
                 tc.tile_pool(name="ps3s", bufs=2, space="PSUM") as ps3s, \
                 tc.tile_pool(name="ps3r", bufs=4, space="PSUM") as ps3r:
                mask_t = mk.tile([128, 4 * 512], F32, tag="mask")
                nc.sync.dma_start(mask_t[:], masks[:])

                for t0 in (0, 512):
                    ssb_list = []
                    for li in range(6):
                        ps = ps3s.tile([128, 512], F32, tag="ps_s")
                        k0 = t0 + 1 + 128 * li
                        for cc in range(4):
                            nc.tensor.matmul(
                                ps[:], kt[cc][:, k0:k0 + 128],
                                qt[cc][:, t0:t0 + 512],
                                start=(cc == 0), stop=(cc == 3),
                                skip_group_check=True)
                        ssb = ssbp.tile([128, 512], F32R, tag="ssb")
                        if li < 4:
                            nc.vector.tensor_mul(
                                ssb[:], ps[:], mask_t[:, 512 * li:512 * (li + 1)])
                        else:
                            nc.vector.tensor_copy(ssb[:], ps[:])
                        ssb_list.append(ssb)
                    for cc in range(4):
                        pr = ps3r.tile([128, 512], F32, tag="ps_r")
                        for li in range(6):
                            nc.tensor.matmul(
                                pr[:],
                                vsc[t0 // 128 + li][:, 128 * cc:128 * (cc + 1)],
                                ssb_list[li][:],
                                start=(li == 0), stop=(li == 5),
                                skip_group_check=True)
                        nc.vector.tensor_copy(rt[cc][:, t0:t0 + 512], pr[:])

            # ---- Phase 4: synthesis out[r, v] = RT.T @ Wo_eff, row-scaled ----
            with tc.tile_pool(name="wo", bufs=2) as wop, \
                 tc.tile_pool(name="ob", bufs=3) as obp, \
                 tc.tile_pool(name="ps4", bufs=2, space="PSUM") as ps4:
                for vg in range(16):
                    wo_t = []
                    for cc in range(4):
                        wt = wop.tile([128, 512], F32R, tag=f"wo{cc}",
                                      name=f"wo{cc}")
                        nc.sync.dma_start(
                            wt[:], woe[128 * cc:128 * (cc + 1),
                                       512 * vg:512 * (vg + 1)])
                        wo_t.append(wt)
                    for rq in range(8):
                        po = ps4.tile([128, 512], F32, tag="po")
                        for cc in range(4):
                            nc.tensor.matmul(
                                po[:], rt[cc][:, 128 * rq:128 * (rq + 1)],
                                wo_t[cc][:], start=(cc == 0), stop=(cc == 3),
                                skip_group_check=True)
                        ot = obp.tile([128, 512], F32, tag="ot")
                        nc.vector.tensor_scalar_mul(ot[:], po[:],
                                                    ovec_t[:, rq:rq + 1])
                        nc.sync.dma_start(
                            out[128 * rq:128 * (rq + 1),
                                512 * vg:512 * (vg + 1)], ot[:])
    nc.compile()
    return nc


def _host_consts(decay, out_scale):
    vv = np.arange(V, dtype=np.float64)
    kk = np.arange(1, NF + 1, dtype=np.float64)
    th = 2.0 * np.pi * np.outer(vv, kk) / V            # [V, NF]
    F = np.empty((V, M2), dtype=np.float64)
    F[:, :NF] = np.cos(th)
    F[:, NF:] = -np.sin(th)
    G = np.empty((M2, V), dtype=np.float64)
    G[:NF, :] = (2.0 / V) * np.cos(th).T
    G[NF:, :] = -(2.0 / V) * np.sin(th).T
    # channel permutation for the rfft fold: [even-re|even-im|odd-re|odd-im]
    jj = np.arange(128)
    perm = np.concatenate([2 * jj + 1, 2 * jj + 257, 2 * jj, 2 * jj + 256])
    F2 = np.ascontiguousarray(F[:V // 2, perm])

    p = np.arange(128, dtype=np.float64)
    dv = np.empty((128, 10), dtype=np.float64)
    for rc in range(10):
        dv[:, rc] = decay ** (1.0 + 128 * rc + p)
    ov = np.empty((128, 8), dtype=np.float64)
    for rq in range(8):
        ov[:, rq] = decay ** (-(128.0 * rq + p) - 1.0) * out_scale

    msk = np.zeros((128, 4 * 512), dtype=np.float32)
    tri = np.tril(np.ones((128, 128), dtype=np.float32))
    for li in range(4):
        blk = msk[:, 512 * li:512 * (li + 1)]
        blk[:, :128 * li] = 1.0
        blk[:, 128 * li:128 * (li + 1)] = tri
    return (F2.astype(np.float32), perm, G, dv.astype(np.float32),
            ov.astype(np.float32), msk)


def kernel(x, wq, wk, wv, wo, decay_logit, out_scale):
    x = np.asarray(x, dtype=np.float32)
    wq = np.asarray(wq, dtype=np.float32)
    wk = np.asarray(wk, dtype=np.float32)
    wv = np.asarray(wv, dtype=np.float32)
    wo = np.asarray(wo, dtype=np.float32)
    decay = 1.0 / (1.0 + np.exp(-float(np.asarray(decay_logit))))
    osc = float(np.asarray(out_scale))

    F2, perm, G, dv, ov, msk = _host_consts(decay, osc)
    woe = np.ascontiguousarray((wo.astype(np.float64) @ G).astype(np.float32))
    wqt = np.ascontiguousarray(wq.T[perm])
    wkt = np.ascontiguousarray(wk.T[perm])
    wvt = np.ascontiguousarray(wv.T[perm])
    idn = np.eye(128, dtype=np.float32)

    in_maps = []
    for c in range(NCORES):
        b, h = c // 2, c % 2
        xsh = np.zeros((KVR, V), dtype=np.float32)
        lo = h * QR
        hi = min(T, lo + KVR)
        xsh[:hi - lo] = x[b, lo:hi]
        in_maps.append({
            "xs": xsh, "f": F2, "wqt": wqt, "wkt": wkt, "wvt": wvt,
            "woe": woe, "idn": idn, "masks": msk, "dvecs": dv, "ovecs": ov,
        })

    if "nc" not in _CACHE:
        _CACHE["nc"] = _build()
    nc = _CACHE["nc"]

    res = bass_utils.run_bass_kernel_spmd(nc, in_maps, core_ids=list(range(NCORES)))
    out = np.empty((B, T, V), dtype=np.float32)
    for c in range(NCORES):
        b, h = c // 2, c % 2
        out[b, h * QR:(h + 1) * QR] = res.results[c]["out"]
    return out



# revision 5
# speedup vs baseline: 1.9508x; 1.9508x over previous
"""GaussMemoryStep Trainium2 kernel (v2).

Math (reference):
  X_ri = rfft(x)[1:257] as [real, imag]            # [B,T,512]
  q,k,v = X_ri @ {wq,wk,wv}.T                      # [B,T,512]
  scores = q @ k^T                                 # [B,T,T]
  weights[i,j] = decay^(j-i-1) for j>i else 0      # future-looking decay band
  retrieved = (scores*weights) @ v
  out = irfft(embed(retrieved @ wo)) * out_scale   # [B,T,8192]

Kernel strategy (8 cores, core c handles sample b=c//2, half h=c%2):
  - Host precomputes the radix-2/4 DFT folds of x (xpp/xpm over 2048 for
    k=0 mod 4 / k=2 mod 4, xm over 4096 for odd k), transposed + packed +
    cast to bf16, so the on-device rfft is three dense bf16 matmul passes
    with no transposes.
  - decay handled exactly via: v rows pre-scaled by d^j (per-partition
    scale), retrieved rows post-scaled by d^-(i+1) * out_scale; attention
    needs only a causal-band mask (band of 768 keys per 512-query block;
    decay^o < 4e-6 beyond 256 offsets).
  - Synthesis in two steps: Y = retrieved @ wo (small), then irfft as a
    matmul against G with the frequency axis split even/odd-k so only the
    first half of the output columns are computed (out[v+4096] = ye - yo).
  - All big operands bf16 (PSUM accumulation fp32); end-to-end rel err vs
    the fp32 reference ~7e-3.
"""
import numpy as np
import ml_dtypes
import concourse.bacc as bacc
import concourse.mybir as mybir
import concourse.tile as tile
from concourse import bass_utils

B, T, V, C, NF = 4, 2048, 8192, 512, 256
QR = 1024              # query rows per core
KVR = 1280             # kv rows per core (query rows + band; h=1 zero-padded)
KVP = 1536             # padded kT free size
NCORES = 8
SP = [(0, 512), (512, 512), (1024, 256)]   # kv row spans
QSP = [(0, 512), (512, 512)]               # query row spans

F32 = mybir.dt.float32
BF16 = mybir.dt.bfloat16
BFNP = ml_dtypes.bfloat16

_CACHE = {}


def _build():
    nc = bacc.Bacc("TRN2", target_bir_lowering=False, debug=False)

    # x folds, v'-chunk-packed: chunk i of [128, KVR] at cols [KVR*i:...]
    xpp = nc.dram_tensor("xpp", [128, 16 * KVR], BF16, kind="ExternalInput").ap()
    xpm = nc.dram_tensor("xpm", [128, 16 * KVR], BF16, kind="ExternalInput").ap()
    xm = nc.dram_tensor("xm", [128, 32 * KVR], BF16, kind="ExternalInput").ap()
    # DFT matrices, chunk-packed along contraction
    f00 = nc.dram_tensor("f00", [128, 16 * 128], BF16, kind="ExternalInput").ap()
    f02 = nc.dram_tensor("f02", [128, 16 * 128], BF16, kind="ExternalInput").ap()
    f1 = nc.dram_tensor("f1", [128, 32 * 256], BF16, kind="ExternalInput").ap()
    # projection weights [in-chan(perm), out], chunk-packed: [128, 4*512]
    wqt = nc.dram_tensor("wqt", [128, 4 * C], BF16, kind="ExternalInput").ap()
    wkt = nc.dram_tensor("wkt", [128, 4 * C], BF16, kind="ExternalInput").ap()
    wvt = nc.dram_tensor("wvt", [128, 4 * C], BF16, kind="ExternalInput").ap()
    # wo with output chans permuted to [even-k re|im, odd-k re|im]
    wo2 = nc.dram_tensor("wo2", [128, 4 * C], BF16, kind="ExternalInput").ap()
    # irfft halves [256, 4096] chunk-packed as [128, 2*4096]
    ge = nc.dram_tensor("ge", [128, 2 * 4096], BF16, kind="ExternalInput").ap()
    go = nc.dram_tensor("go", [128, 2 * 4096], BF16, kind="ExternalInput").ap()
    # f32 consts: masks [128,2048] | row-scale [128,1024] | dvec [128,16]
    consts = nc.dram_tensor("consts", [128, 2048 + QR + 16], F32,
                            kind="ExternalInput").ap()
    out = nc.dram_tensor("out", [QR, V], F32, kind="ExternalOutput").ap()

    with tile.TileContext(nc) as tc:
      with tc.tile_pool(name="ct", bufs=1) as ctp, \
           tc.tile_pool(name="yrt", bufs=1) as yrtp:
        cons = ctp.tile([128, 2048 + QR + 16], F32, tag="cons")
        nc.sync.dma_start(cons[:], consts[:])
        mask_t = cons[:, 0:2048]
        stile_t = cons[:, 2048:2048 + QR]
        dvec_t = cons[:, 2048 + QR:2048 + QR + 16]

        rt = [yrtp.tile([128, QR], BF16, tag=f"rt{i}", name=f"rt{i}")
              for i in range(4)]
        yt = [yrtp.tile([128, QR], BF16, tag=f"yt{i}", name=f"yt{i}")
              for i in range(4)]

        with tc.tile_pool(name="xri", bufs=1) as xrip:
            xri = [xrip.tile([128, KVP], BF16, tag=f"xri{i}", name=f"xri{i}")
                   for i in range(4)]
            for i in range(4):
                nc.vector.memset(xri[i][:, KVR:KVP], 0.0)

            # ---- Phase 1: folded DFT -> X_riT [512 chans, 1280 rows] ----
            with tc.tile_pool(name="fp", bufs=1) as fp, \
                 tc.tile_pool(name="xs", bufs=2) as xsp, \
                 tc.tile_pool(name="ps1", bufs=1, space="PSUM") as ps1:
                f00t = fp.tile([128, 16 * 128], BF16, tag="f00")
                nc.sync.dma_start(f00t[:], f00[:])
                f02t = fp.tile([128, 16 * 128], BF16, tag="f02")
                nc.sync.dma_start(f02t[:], f02[:])
                f1t = fp.tile([128, 32 * 256], BF16, tag="f1")
                nc.sync.dma_start(f1t[:], f1[:])

                GRP = 4  # x chunks per DMA
                passes = [(xpp, 16, f00t, 128, [0]),
                          (xpm, 16, f02t, 128, [1]),
                          (xm, 32, f1t, 256, [2, 3])]
                for (xd, nch, ft, fw, ccs) in passes:
                    pst = {}
                    for ci in range(len(ccs)):
                        for si, (s0, sl) in enumerate(SP):
                            pst[(ci, si)] = ps1.tile(
                                [128, sl], F32, tag=f"s{ci}{si}",
                                name=f"ps1_{ci}{si}")
                    for g in range(nch // GRP):
                        xt = xsp.tile([128, GRP * KVR], BF16, tag="xt")
                        nc.sync.dma_start(
                            xt[:], xd[:, g * GRP * KVR:(g + 1) * GRP * KVR])
                        for j in range(GRP):
                            i = g * GRP + j
                            for ci, cc in enumerate(ccs):
                                lhsT = ft[:, fw * i + 128 * ci:
                                          fw * i + 128 * (ci + 1)]
                                for si, (s0, sl) in enumerate(SP):
                                    nc.tensor.matmul(
                                        pst[(ci, si)][:], lhsT,
                                        xt[:, KVR * j + s0:KVR * j + s0 + sl],
                                        start=(i == 0), stop=(i == nch - 1),
                                        skip_group_check=True)
                    for ci, cc in enumerate(ccs):
                        for si, (s0, sl) in enumerate(SP):
                            nc.vector.tensor_copy(xri[cc][:, s0:s0 + sl],
                                                  pst[(ci, si)][:])

            # ---- Phase 2+3: projections, attention, Y = rt @ wo2 ----
            with tc.tile_pool(name="wp", bufs=1) as wp, \
                 tc.tile_pool(name="prj", bufs=1) as prj:
                wq_t = wp.tile([128, 4 * C], BF16, tag="wq")
                nc.sync.dma_start(wq_t[:], wqt[:])
                wk_t = wp.tile([128, 4 * C], BF16, tag="wk")
                nc.sync.dma_start(wk_t[:], wkt[:])
                wv_t = wp.tile([128, 4 * C], BF16, tag="wv")
                nc.sync.dma_start(wv_t[:], wvt[:])
                wo_t = wp.tile([128, 4 * C], BF16, tag="wo")
                nc.sync.dma_start(wo_t[:], wo2[:])

                qt = [prj.tile([128, QR], BF16, tag=f"qt{cc}", name=f"qt{cc}")
                      for cc in range(4)]
                kt = [prj.tile([128, KVP], BF16, tag=f"kt{cc}", name=f"kt{cc}")
                      for cc in range(4)]
                vsc = [prj.tile([128, C], BF16, tag=f"vs{rc}", name=f"vs{rc}")
                       for rc in range(10)]
                for cc in range(4):
                    nc.vector.memset(kt[cc][:, KVR:KVP], 0.0)

                with tc.tile_pool(name="ps2", bufs=4, space="PSUM") as ps2:
                    for cc in range(4):
                        for (s0, sl) in QSP:
                            ps = ps2.tile([128, 512], F32, tag="pp")
                            for mc in range(4):
                                nc.tensor.matmul(
                                    ps[:],
                                    wq_t[:, 512 * mc + 128 * cc:
                                         512 * mc + 128 * (cc + 1)],
                                    xri[mc][:, s0:s0 + sl],
                                    start=(mc == 0), stop=(mc == 3),
                                    skip_group_check=True)
                            nc.vector.tensor_copy(qt[cc][:, s0:s0 + sl], ps[:])
                        for (s0, sl) in SP:
                            ps = ps2.tile([128, 512], F32, tag="pp")
                            for mc in range(4):
                                nc.tensor.matmul(
                                    ps[:, :sl],
                                    wk_t[:, 512 * mc + 128 * cc:
                                         512 * mc + 128 * (cc + 1)],
                                    xri[mc][:, s0:s0 + sl],
                                    start=(mc == 0), stop=(mc == 3),
                                    skip_group_check=True)
                            nc.vector.tensor_copy(kt[cc][:, s0:s0 + sl],
                                                  ps[:, :sl])
                    for rc in range(10):
                        ps = ps2.tile([128, 512], F32, tag="pp")
                        for mc in range(4):
                            nc.tensor.matmul(
                                ps[:], xri[mc][:, 1 + 128 * rc:129 + 128 * rc],
                                wv_t[:, 512 * mc:512 * (mc + 1)],
                                start=(mc == 0), stop=(mc == 3),
                                skip_group_check=True)
                        nc.vector.tensor_scalar_mul(vsc[rc][:], ps[:],
                                                    dvec_t[:, rc:rc + 1])

                # banded decay attention + wo projection
                with tc.tile_pool(name="sb", bufs=8) as sbp, \
                     tc.tile_pool(name="ps3s", bufs=2, space="PSUM") as ps3s, \
                     tc.tile_pool(name="ps3r", bufs=2, space="PSUM") as ps3r, \
                     tc.tile_pool(name="ps3y", bufs=2, space="PSUM") as ps3y:
                    for t0 in (0, 512):
                        ssb = []
                        for li in range(6):
                            ps = ps3s.tile([128, 512], F32, tag="ss")
                            k0 = t0 + 1 + 128 * li
                            for cc in range(4):
                                nc.tensor.matmul(
                                    ps[:], kt[cc][:, k0:k0 + 128],
                                    qt[cc][:, t0:t0 + 512],
                                    start=(cc == 0), stop=(cc == 3),
                                    skip_group_check=True)
                            sb = sbp.tile([128, 512], BF16, tag="sb")
                            if li < 4:
                                nc.vector.tensor_mul(
                                    sb[:], ps[:],
                                    mask_t[:, 512 * li:512 * (li + 1)])
                            else:
                                nc.vector.tensor_copy(sb[:], ps[:])
                            ssb.append(sb)
                        for cc in range(4):
                            pr = ps3r.tile([128, 512], F32, tag="pr")
                            for li in range(6):
                                nc.tensor.matmul(
                                    pr[:],
                                    vsc[t0 // 128 + li][:, 128 * cc:
                                                        128 * (cc + 1)],
                                    ssb[li][:],
                                    start=(li == 0), stop=(li == 5),
                                    skip_group_check=True)
                            nc.vector.tensor_mul(rt[cc][:, t0:t0 + 512], pr[:],
                                                 stile_t[:, t0:t0 + 512])
                        for yc in range(4):
                            ps = ps3y.tile([128, 512], F32, tag="yy")
                            for mc in range(4):
                                nc.tensor.matmul(
                                    ps[:],
                                    wo_t[:, 512 * mc + 128 * yc:
                                         512 * mc + 128 * (yc + 1)],
                                    rt[mc][:, t0:t0 + 512],
                                    start=(mc == 0), stop=(mc == 3),
                                    skip_group_check=True)
                            nc.vector.tensor_copy(yt[yc][:, t0:t0 + 512], ps[:])

        # ---- Phase 4: out = [ye+yo | ye-yo], ye = ytE @ Ge, yo = ytO @ Go
        with tc.tile_pool(name="gp", bufs=1) as gp, \
             tc.tile_pool(name="ob", bufs=2) as obp, \
             tc.tile_pool(name="oc", bufs=4) as ocp, \
             tc.tile_pool(name="ps4e", bufs=3, space="PSUM") as ps4e, \
             tc.tile_pool(name="ps4o", bufs=3, space="PSUM") as ps4o:
            ge_t = gp.tile([128, 2 * 4096], BF16, tag="ge")
            nc.sync.dma_start(ge_t[:], ge[:])
            go_t = gp.tile([128, 2 * 4096], BF16, tag="go")
            nc.sync.dma_start(go_t[:], go[:])
            for rq in range(8):
                for vh in range(2):
                    ot = obp.tile([128, 4096], F32, tag="ot")
                    for vq in range(4):
                        vg = 4 * vh + vq
                        pe_ = ps4e.tile([128, 512], F32, tag="pe")
                        po_ = ps4o.tile([128, 512], F32, tag="po")
                        for ci in range(2):
                            nc.tensor.matmul(
                                pe_[:], yt[ci][:, 128 * rq:128 * (rq + 1)],
                                ge_t[:, 4096 * ci + 512 * vg:
                                     4096 * ci + 512 * (vg + 1)],
                                start=(ci == 0), stop=(ci == 1),
                                skip_group_check=True)
                            nc.tensor.matmul(
                                po_[:], yt[2 + ci][:, 128 * rq:128 * (rq + 1)],
                                go_t[:, 4096 * ci + 512 * vg:
                                     4096 * ci + 512 * (vg + 1)],
                                start=(ci == 0), stop=(ci == 1),
                                skip_group_check=True)
                        # DVE can't take two PSUM operands: stage yo in SBUF
                        yo_c = ocp.tile([128, 512], F32, tag="yoc",
                                        name="yo_c")
                        nc.vector.tensor_copy(yo_c[:], po_[:])
                        nc.vector.tensor_add(
                            ot[:, 512 * vq:512 * (vq + 1)], pe_[:], yo_c[:])
                        nc.vector.tensor_sub(
                            ot[:, 2048 + 512 * vq:2048 + 512 * (vq + 1)],
                            pe_[:], yo_c[:])
                    # cols [0:2048] -> v in [2048*vh, ...); [2048:4096] -> +4096
                    nc.sync.dma_start(
                        out[128 * rq:128 * (rq + 1),
                            2048 * vh:2048 * (vh + 1)], ot[:, 0:2048])
                    nc.sync.dma_start(
                        out[128 * rq:128 * (rq + 1),
                            4096 + 2048 * vh:4096 + 2048 * (vh + 1)],
                        ot[:, 2048:4096])
    nc.compile()
    return nc


def _chunk_pack(a, nch, w):
    # [nch*128, w] -> [128, nch*w] with chunk i at cols [w*i : w*(i+1)]
    return np.ascontiguousarray(
        a.reshape(nch, 128, w).transpose(1, 0, 2).reshape(128, nch * w))


def _host_consts(decay, osc):
    def fmat(ks, L):
        v = np.arange(L, dtype=np.float64)
        th = 2.0 * np.pi * np.outer(v, ks) / float(V)
        return np.concatenate([np.cos(th), -np.sin(th)], axis=1)

    k0 = np.arange(4, 257, 4)
    k2 = np.arange(2, 256, 4)
    k1 = np.arange(1, 256, 2)
    F00 = _chunk_pack(fmat(k0, 2048).astype(BFNP), 16, 128)
    F02 = _chunk_pack(fmat(k2, 2048).astype(BFNP), 16, 128)
    F1 = _chunk_pack(fmat(k1, 4096).astype(BFNP), 32, 256)
    # X_ri channel order: [k0 re|im, k2 re|im, k1 re|im]
    perm = np.concatenate([k0 - 1, 256 + k0 - 1, k2 - 1, 256 + k2 - 1,
                           k1 - 1, 256 + k1 - 1])
    # Y channel order: [even-k re, even-k im, odd-k re, odd-k im]
    ke = np.arange(2, 257, 2)
    ko = np.arange(1, 256, 2)
    perm2 = np.concatenate([ke - 1, 256 + ke - 1, ko - 1, 256 + ko - 1])

    vv = np.arange(V, dtype=np.float64)
    kk = np.arange(1, NF + 1, dtype=np.float64)
    th = 2.0 * np.pi * np.outer(vv, kk) / V
    G = np.concatenate([(2.0 / V) * np.cos(th).T,
                        -(2.0 / V) * np.sin(th).T], axis=0)  # [512, V]
    Gp = G[perm2]
    Ge = _chunk_pack(Gp[:256, :4096].astype(BFNP), 2, 4096)
    Go = _chunk_pack(Gp[256:, :4096].astype(BFNP), 2, 4096)

    msk = np.zeros((128, 4 * 512), dtype=np.float32)
    tri = np.tril(np.ones((128, 128), dtype=np.float32))
    for li in range(4):
        blk = msk[:, 512 * li:512 * (li + 1)]
        blk[:, :128 * li] = 1.0
        blk[:, 128 * li:128 * (li + 1)] = tri

    p = np.arange(128, dtype=np.float64)
    stile = np.tile((decay ** (-(np.arange(QR, dtype=np.float64)) - 1.0)
                     * osc).astype(np.float32)[None, :], (128, 1))
    dv = np.zeros((128, 16), dtype=np.float32)
    for rc in range(10):
        dv[:, rc] = (decay ** (1.0 + 128 * rc + p)).astype(np.float32)
    consts = np.ascontiguousarray(
        np.concatenate([msk, stile, dv], axis=1).astype(np.float32))
    return F00, F02, F1, perm, perm2, Ge, Go, consts


def build_in_maps(x, wq, wk, wv, wo, decay_logit, out_scale):
    x = np.asarray(x, dtype=np.float32)
    wq = np.asarray(wq, dtype=np.float32)
    wk = np.asarray(wk, dtype=np.float32)
    wv = np.asarray(wv, dtype=np.float32)
    wo = np.asarray(wo, dtype=np.float32)
    decay = 1.0 / (1.0 + np.exp(-float(np.asarray(decay_logit))))
    osc = float(np.asarray(out_scale))

    F00, F02, F1, perm, perm2, Ge, Go, consts = _host_consts(decay, osc)
    wqt = _chunk_pack(np.ascontiguousarray(wq[:, perm].T).astype(BFNP), 4, C)
    wkt = _chunk_pack(np.ascontiguousarray(wk[:, perm].T).astype(BFNP), 4, C)
    wvt = _chunk_pack(np.ascontiguousarray(wv[:, perm].T).astype(BFNP), 4, C)
    wo2 = _chunk_pack(wo[:, perm2].astype(BFNP), 4, C)

    in_maps = []
    for c in range(NCORES):
        b, h = c // 2, c % 2
        lo = h * QR
        hi = min(T, lo + KVR)
        xs = np.zeros((KVR, V), dtype=np.float32)
        xs[:hi - lo] = x[b, lo:hi]
        xp = xs[:, :4096] + xs[:, 4096:]
        xmf = (xs[:, :4096] - xs[:, 4096:]).astype(BFNP)
        xppf = (xp[:, :2048] + xp[:, 2048:]).astype(BFNP)
        xpmf = (xp[:, :2048] - xp[:, 2048:]).astype(BFNP)
        in_maps.append({
            "xpp": _chunk_pack(np.ascontiguousarray(xppf.T), 16, KVR),
            "xpm": _chunk_pack(np.ascontiguousarray(xpmf.T), 16, KVR),
            "xm": _chunk_pack(np.ascontiguousarray(xmf.T), 32, KVR),
            "f00": F00, "f02": F02, "f1": F1,
            "wqt": wqt, "wkt": wkt, "wvt": wvt, "wo2": wo2,
            "ge": Ge, "go": Go, "consts": consts,
        })
    return in_maps


def kernel(x, wq, wk, wv, wo, decay_logit, out_scale):
    in_maps = build_in_maps(x, wq, wk, wv, wo, decay_logit, out_scale)
    if "nc" not in _CACHE:
        _CACHE["nc"] = _build()
    nc = _CACHE["nc"]
    res = bass_utils.run_bass_kernel_spmd(nc, in_maps,
                                          core_ids=list(range(NCORES)))
    out = np.empty((B, T, V), dtype=np.float32)
    for c in range(NCORES):
        b, h = c // 2, c % 2
        out[b, h * QR:(h + 1) * QR] = res.results[c]["out"]
    return out
